# revision 1
# baseline (speedup 1.0000x reference)
"""Trainium2 Bass kernel for ContactDiffusion GNN message passing.

out = latent + K_norm @ msg,  K = (D+eps)^(-alpha_ij) * exp(-D/12), row-normalized,
msg = MLP(latent).

Strategy (8 NeuronCores, SPMD, full inputs in / full output out):
 - Host: KD-sort points spatially; each core owns 1024 contiguous sorted rows.
 - Device per core: pairwise d2 for its [8192 x 1024] K^T slab via a Gram-form
   fp16-split feature matmul (k=18), elementwise K chain on ScalarE
   (ln / exp, single activation-table set), contraction + row-sums on PE.
 - The core's own diagonal block is computed exactly (ACT Square with
   per-partition bias = direct (ci-cj)^2) with exact ln(D+eps); the Gram pass
   suppresses that block via a rank-1 indicator feature.
 - Cross-core close pairs ("stragglers", d2 < 0.09) are deterministically
   suppressed on device via a second rank-1 indicator feature and their exact
   contribution is added back on host using the exported row sums.
 - MLP is sharded (each core computes msg for its rows); msg is AllGathered.
"""

import math
import os
import sys
from contextlib import ExitStack

import numpy as np

sys.path.insert(0, "/opt/trn_rl_repo")

import ml_dtypes

import concourse.bass as bass
import concourse.tile as tile
from concourse import bacc, mybir
from concourse.bass_utils import run_bass_kernel_spmd

F32 = mybir.dt.float32
F16 = mybir.dt.float16
BF16 = mybir.dt.bfloat16
AF = mybir.ActivationFunctionType
ALU = mybir.AluOpType

NP_BF16 = ml_dtypes.bfloat16

N, DIM, NCORE = 8192, 512, 8
NSH = N // NCORE            # rows per core
EPS, LAM = 1e-4, 12.0
TSTRAG = 0.09               # d2 below this across cores -> straggler
SUP = 1e3                   # suppressor feature magnitude (SUP^2 added to d2)
GROUP = 16                  # j-tiles per psum_out accumulation group
LN12 = math.log(12.0)

_BUILT = {}


# ----------------------------------------------------------------------------
# device program
# ----------------------------------------------------------------------------
def build_program(n=N, dim=DIM, nsh=NSH, group=GROUP, trace_sim=False, gelu=True, taps=False):
    nt_own = nsh // 128          # own-block j-tiles
    nt_main = n // 128           # main-pass j-tiles
    n_kd = dim // 128            # contraction k-blocks for MLP
    n_ic = nsh // 128            # i-chunks
    nt_all = nt_own + nt_main

    nc = bacc.Bacc("TRN2", target_bir_lowering=False, debug=False,
                   num_devices=NCORE)

    # ---- dram params ----
    featj = nc.dram_tensor("featj", [18, n], F16, kind="ExternalInput").ap()
    feati = nc.dram_tensor("feati", [18, nsh], F16, kind="ExternalInput").ap()
    ahj = nc.dram_tensor("ahj", [128, nt_main], F32, kind="ExternalInput").ap()
    ahjo = nc.dram_tensor("ahjo", [128, nt_own], F32, kind="ExternalInput").ap()
    ahibc = nc.dram_tensor("ahibc", [128, nsh], F32, kind="ExternalInput").ap()
    cib = nc.dram_tensor("cib", [128, 3 * nsh], F32, kind="ExternalInput").ap()
    ncjo = nc.dram_tensor("ncjo", [128, 3 * nt_own], F32, kind="ExternalInput").ap()
    latT = nc.dram_tensor("latT", [dim, nsh], F16, kind="ExternalInput").ap()
    w1t = nc.dram_tensor("w1t", [dim, dim], F16, kind="ExternalInput").ap()
    w2t = nc.dram_tensor("w2t", [dim, dim], F16, kind="ExternalInput").ap()
    b1c = nc.dram_tensor("b1c", [128, n_kd], F32, kind="ExternalInput").ap()
    b2r = nc.dram_tensor("b2r", [1, dim], F16, kind="ExternalInput").ap()
    onescol = nc.dram_tensor("onescol", [1, 128], F16, kind="ExternalInput").ap()
    ones128 = nc.dram_tensor("ones128", [128, 1], BF16, kind="ExternalInput").ap()

    num_out = nc.dram_tensor("num", [nsh, dim], F32, kind="ExternalOutput").ap()
    tap_aps = {}
    if taps:
        for tn in ["tap_d2", "tap_l", "tap_d12", "tap_t", "tap_k", "tap_kown"]:
            tap_aps[tn] = nc.dram_tensor(tn, [128, nsh], F32, kind="ExternalOutput").ap()
    srow_out = nc.dram_tensor("srow", [128, n_ic], F32, kind="ExternalOutput").ap()

    with tile.TileContext(nc, trace_sim=trace_sim) as tc, ExitStack() as ctx:
        pers = ctx.enter_context(tc.tile_pool(name="pers", bufs=1))
        p_big = ctx.enter_context(tc.tile_pool(name="pbig", bufs=2, space="PSUM"))
        p_out = ctx.enter_context(tc.tile_pool(name="pout", bufs=2, space="PSUM"))
        p_s = ctx.enter_context(tc.tile_pool(name="ps", bufs=1, space="PSUM"))
        sq_pool = ctx.enter_context(tc.tile_pool(name="sq", bufs=1))
        l_pool = ctx.enter_context(tc.tile_pool(name="lp", bufs=2))
        d12_pool = ctx.enter_context(tc.tile_pool(name="d12", bufs=2))
        amt_pool = ctx.enter_context(tc.tile_pool(name="amt", bufs=2))
        k_pool = ctx.enter_context(tc.tile_pool(name="kp", bufs=group + 4))
        kraw_pool = ctx.enter_context(tc.tile_pool(name="kraw", bufs=2))
        msg_pool = ctx.enter_context(tc.tile_pool(name="msgp", bufs=group + 4))
        dram = ctx.enter_context(tc.tile_pool(name="dram", bufs=1, space="DRAM"))
        tapp = ctx.enter_context(tc.tile_pool(name="tapp", bufs=2)) if taps else None

        dma = nc.sync.dma_start

        # ---- persistent SBUF loads ----
        featj_sb = pers.tile([18, n], F16)
        dma(featj_sb[:], featj[:])
        feati_sb = pers.tile([18, nsh], F16)
        dma(feati_sb[:], feati[:])
        ahj_sb = pers.tile([128, nt_main], F32)
        dma(ahj_sb[:], ahj[:])
        ahjo_sb = pers.tile([128, nt_own], F32)
        dma(ahjo_sb[:], ahjo[:])
        ahibc_sb = pers.tile([128, nsh], F32)
        dma(ahibc_sb[:], ahibc[:])
        cib_sb = pers.tile([128, 3 * nsh], F32)
        dma(cib_sb[:], cib[:])
        ncjo_sb = pers.tile([128, 3 * nt_own], F32)
        dma(ncjo_sb[:], ncjo[:])
        b1c_sb = pers.tile([128, n_kd], F32)
        dma(b1c_sb[:], b1c[:])
        b2r_sb = pers.tile([1, dim], F16)
        dma(b2r_sb[:], b2r[:])
        onescol_sb = pers.tile([1, 128], F16)
        dma(onescol_sb[:], onescol[:])
        ones128_sb = pers.tile([128, 1], BF16)
        dma(ones128_sb[:], ones128[:])
        latT_sb = [pers.tile([128, nsh], F16, tag=f"latT{k}", name=f"latT{k}") for k in range(n_kd)]
        for k in range(n_kd):
            dma(latT_sb[k][:], latT[k * 128:(k + 1) * 128, :])
        w1t_sb = [pers.tile([128, dim], F16, tag=f"w1t{k}", name=f"w1t{k}") for k in range(n_kd)]
        w2t_sb = [pers.tile([128, dim], F16, tag=f"w2t{k}", name=f"w2t{k}") for k in range(n_kd)]
        for k in range(n_kd):
            dma(w1t_sb[k][:], w1t[k * 128:(k + 1) * 128, :])
            dma(w2t_sb[k][:], w2t[k * 128:(k + 1) * 128, :])

        acc = pers.tile([128, n_ic * dim], F32)       # out accumulators
        nc.vector.memset(acc[:], 0.0)

        bias_ln12 = pers.tile([128, 1], F32)
        nc.gpsimd.memset(bias_ln12[:], -LN12)
        bias_eps = pers.tile([128, 1], F32)
        nc.gpsimd.memset(bias_eps[:], EPS)
        bias_ln6 = pers.tile([128, 1], F32)
        nc.gpsimd.memset(bias_ln6[:], -math.log(6.0))

        msgown_d = dram.tile([nsh, dim], BF16)
        msgall_d = dram.tile([n, dim], BF16)

        # ---- phase A: MLP (gelu table set) ----
        cw = min(512, nsh)
        hT_sb = [pers.tile([128, nsh], F16, tag=f"hT{k}", name=f"hT{k}") for k in range(n_kd)]
        for mc in range(n_kd):
            ph = p_big.tile([128, nsh], F32, tag="big", name="ph")
            for half in range(nsh // cw):
                hs = slice(half * cw, (half + 1) * cw)
                for kb in range(n_kd):
                    nc.tensor.matmul(
                        ph[:, hs],
                        lhsT=w1t_sb[kb][:, mc * 128:(mc + 1) * 128],
                        rhs=latT_sb[kb][:, hs],
                        start=(kb == 0), stop=(kb == n_kd - 1))
            nc.scalar.activation(hT_sb[mc][:], ph[:], AF.Gelu if gelu else AF.Identity,
                                 bias=b1c_sb[:, mc:mc + 1], scale=1.0)

        msgown_sb = [pers.tile([128, dim], BF16, tag=f"mo{ic}", name=f"mo{ic}") for ic in range(n_ic)]
        for ic in range(n_ic):
            pm = p_out.tile([128, dim], F32, tag="out", name="pm")
            for kb in range(n_kd):
                nc.tensor.matmul(
                    pm[:],
                    lhsT=hT_sb[kb][:, ic * 128:(ic + 1) * 128],
                    rhs=w2t_sb[kb][:],
                    start=(kb == 0), stop=False)
            nc.tensor.matmul(pm[:], lhsT=onescol_sb[:], rhs=b2r_sb[:],
                             start=False, stop=True)
            nc.scalar.copy(msgown_sb[ic][:], pm[:])
            dma(msgown_d[ic * 128:(ic + 1) * 128, :], msgown_sb[ic][:])

        # ---- phase B: AllGather msg ----
        nc.gpsimd.collective_compute(
            "AllGather", ALU.bypass,
            ins=[msgown_d.opt()], outs=[msgall_d.opt()],
            replica_groups=[list(range(NCORE))])

        # ---- phase C/D: slab loop ----
        ps_s = p_s.tile([128, n_ic], F32)

        def emit_elementwise(jt):
            """produce K tile [128, nsh] bf16 + its msg rhs tile; return both"""
            if jt < nt_own:
                # own-block exact pass
                t = jt
                sqs = []
                for d in range(3):
                    sq = sq_pool.tile([128, nsh], F32, tag=f"sq{d}")
                    nc.scalar.activation(
                        sq[:], cib_sb[:, d * nsh:(d + 1) * nsh], AF.Square,
                        bias=ncjo_sb[:, (t * 3 + d):(t * 3 + d) + 1], scale=1.0)
                    sqs.append(sq)
                nc.vector.tensor_tensor(sqs[0][:], sqs[0][:], sqs[1][:], op=ALU.add)
                nc.vector.tensor_tensor(sqs[0][:], sqs[0][:], sqs[2][:], op=ALU.add)
                l = l_pool.tile([128, nsh], F32)
                nc.scalar.activation(l[:], sqs[0][:], AF.Ln)
                d12 = d12_pool.tile([128, nsh], F32)
                nc.scalar.activation(d12[:], l[:], AF.Exp, bias=bias_ln12[:, 0:1], scale=0.5)
                bigL = amt_pool.tile([128, nsh], F32, tag="bigL")
                nc.scalar.activation(bigL[:], d12[:], AF.Ln, bias=bias_eps[:, 0:1], scale=12.0)
                al = amt_pool.tile([128, nsh], F32, tag="alpha")
                nc.vector.tensor_scalar_add(al[:], ahibc_sb[:], ahjo_sb[:, t:t + 1])
                m = amt_pool.tile([128, nsh], F32, tag="m")
                nc.vector.tensor_tensor(m[:], al[:], bigL[:], op=ALU.mult)
                tt = amt_pool.tile([128, nsh], F32, tag="t")
                nc.gpsimd.tensor_tensor(tt[:], m[:], d12[:], op=ALU.add)
                kraw = kraw_pool.tile([128, nsh], BF16)
                nc.scalar.activation(kraw[:], tt[:], AF.Exp, scale=-1.0)
                ktile = k_pool.tile([128, nsh], BF16)
                nc.gpsimd.affine_select(
                    ktile[:], kraw[:], pattern=[[1, nsh]],
                    compare_op=ALU.not_equal, fill=0.0,
                    base=-(t * 128), channel_multiplier=-1)
                if taps and t == 0:
                    tapk = tapp.tile([128, nsh], F32, tag="tap", name="tapkown")
                    nc.scalar.copy(tapk[:], ktile[:])
                    dma(tap_aps["tap_kown"][:], tapk[:])
                return ktile, msgown_sb[t]
            # main pass (gram)
            t = jt - nt_own
            pd2 = p_big.tile([128, nsh], F32, tag="big", name="pd2")
            for half in range(nsh // cw):
                hs = slice(half * cw, (half + 1) * cw)
                nc.tensor.matmul(pd2[:, hs],
                                 lhsT=featj_sb[:, t * 128:(t + 1) * 128],
                                 rhs=feati_sb[:, hs],
                                 start=True, stop=True)
            l = l_pool.tile([128, nsh], F32)
            nc.scalar.activation(l[:], pd2[:], AF.Ln)
            d12 = d12_pool.tile([128, nsh], F32)
            nc.scalar.activation(d12[:], l[:], AF.Exp, bias=bias_ln6[:, 0:1], scale=0.5)
            al = amt_pool.tile([128, nsh], F32, tag="alpha")
            nc.vector.tensor_scalar_add(al[:], ahibc_sb[:], ahj_sb[:, t:t + 1])
            m = amt_pool.tile([128, nsh], F32, tag="m")
            nc.vector.tensor_tensor(m[:], al[:], l[:], op=ALU.mult)
            tt = amt_pool.tile([128, nsh], F32, tag="t")
            nc.gpsimd.tensor_tensor(tt[:], m[:], d12[:], op=ALU.add)
            ktile = k_pool.tile([128, nsh], BF16)
            nc.scalar.activation(ktile[:], tt[:], AF.Exp, scale=-0.5)
            if taps and t == 8:
                for nm, src in [("tap_d2", pd2), ("tap_l", l), ("tap_d12", d12), ("tap_t", tt)]:
                    tp = tapp.tile([128, nsh], F32, tag="tap", name=f"tp{nm}")
                    nc.scalar.copy(tp[:], src[:])
                    dma(tap_aps[nm][:], tp[:])
                tpk = tapp.tile([128, nsh], F32, tag="tap", name="tpk")
                nc.scalar.copy(tpk[:], ktile[:])
                dma(tap_aps["tap_k"][:], tpk[:])
            mt = msg_pool.tile([128, dim], BF16)
            dma(mt[:], msgall_d[t * 128:(t + 1) * 128, :])
            return ktile, mt

        jt = 0
        while jt < nt_all:
            g = min(group, nt_all - jt)
            tiles = [emit_elementwise(jt + i) for i in range(g)]
            # row-sum matmuls (persistent psum_s accumulation)
            for i, (kt, _) in enumerate(tiles):
                for ic in range(n_ic):
                    nc.tensor.matmul(
                        ps_s[:, ic:ic + 1],
                        lhsT=kt[:, ic * 128:(ic + 1) * 128],
                        rhs=ones128_sb[:],
                        start=(jt + i == 0 and ic == 0),
                        stop=(jt + i == nt_all - 1))
            # contraction for this group
            for ic in range(n_ic):
                po = p_out.tile([128, dim], F32, tag="out", name="po")
                for i, (kt, mt) in enumerate(tiles):
                    nc.tensor.matmul(
                        po[:],
                        lhsT=kt[:, ic * 128:(ic + 1) * 128],
                        rhs=mt[:],
                        start=(i == 0), stop=(i == g - 1))
                asl = slice(ic * dim, (ic + 1) * dim)
                nc.vector.tensor_tensor(acc[:, asl], acc[:, asl], po[:], op=ALU.add)
            jt += g

        # ---- epilogue ----
        ssb = pers.tile([128, n_ic], F32)
        nc.scalar.copy(ssb[:], ps_s[:])
        dma(srow_out[:], ssb[:])
        for ic in range(n_ic):
            dma(num_out[ic * 128:(ic + 1) * 128, :],
                acc[:, ic * dim:(ic + 1) * dim])

    nc.compile()
    return nc


# ----------------------------------------------------------------------------
# host-side preprocessing
# ----------------------------------------------------------------------------
def _kdsort(coords, nblocks):
    def rec(idx, nb):
        if nb == 1:
            return [idx]
        pts = coords[idx]
        ax = int(np.argmax(pts.max(0) - pts.min(0)))
        order = np.argsort(pts[:, ax], kind="stable")
        half = len(idx) // 2
        return rec(idx[order[:half]], nb // 2) + rec(idx[order[half:]], nb // 2)

    return np.concatenate(rec(np.arange(coords.shape[0]), nblocks))


def _split16(x):
    hi = x.astype(np.float16).astype(np.float32)
    lo = (x - hi).astype(np.float16).astype(np.float32)
    return hi, lo


_erf = np.vectorize(math.erf)


def kernel(latent, coords, alpha, W1, b1, W2, b2):
    latent = np.asarray(latent, np.float32)
    coords = np.asarray(coords, np.float32)
    alpha = np.asarray(alpha, np.float32)
    W1 = np.asarray(W1, np.float32)
    b1 = np.asarray(b1, np.float32)
    W2 = np.asarray(W2, np.float32)
    b2 = np.asarray(b2, np.float32)

    perm = _kdsort(coords.astype(np.float64), 64)
    cs = coords[perm]
    als = alpha[perm]
    lats = latent[perm]
    c64 = cs.astype(np.float64)

    core_of = np.arange(N) // NSH
    # stragglers: cross-core pairs with d2 < TSTRAG
    Jstar = [set() for _ in range(NCORE)]
    Istar = [set() for _ in range(NCORE)]
    for i0 in range(0, N, 1024):
        blk = cs[i0:i0 + 1024].astype(np.float64)
        d2b = ((blk[:, None, :] - c64[None, :, :]) ** 2).sum(-1)
        d2b[np.arange(1024), np.arange(i0, i0 + 1024)] = np.inf
        ii, jj = np.nonzero(d2b < TSTRAG)
        ii = ii + i0
        msk = core_of[ii] != core_of[jj]
        for a, b in zip(ii[msk], jj[msk]):
            c = core_of[a]
            Jstar[c].add(int(b))
            Istar[c].add(int(a - c * NSH))

    r = (c64 ** 2).sum(-1).astype(np.float32)
    a2 = (-2.0 * cs).astype(np.float32)
    chj = [_split16(cs[:, d]) for d in range(3)]
    ahi = [_split16(a2[:, d]) for d in range(3)]
    rj = _split16(r)

    in_maps = []
    for core in range(NCORE):
        blk = slice(core * NSH, (core + 1) * NSH)
        rows_j, rows_i = [], []
        for d in range(3):
            for (jp, ip) in [(chj[d][0], ahi[d][0]), (chj[d][0], ahi[d][1]),
                             (chj[d][1], ahi[d][0]), (chj[d][1], ahi[d][1])]:
                rows_j.append(jp)
                rows_i.append(ip[blk])
        ones = np.ones(N, np.float32)
        onesi = np.ones(NSH, np.float32)
        rows_j += [rj[0], rj[1]]
        rows_i += [onesi, onesi]
        rows_j += [ones, ones]
        rows_i += [rj[0][blk], rj[1][blk]]
        mown = np.zeros(N, np.float32)
        mown[blk] = SUP
        rows_j += [mown]
        rows_i += [np.full(NSH, SUP, np.float32)]
        g = np.zeros(N, np.float32)
        h = np.zeros(NSH, np.float32)
        for j in Jstar[core]:
            g[j] = SUP
        for i in Istar[core]:
            h[i] = SUP
        rows_j += [g]
        rows_i += [h]
        featj = np.stack(rows_j).astype(np.float16)
        feati = np.stack(rows_i).astype(np.float16)

        ah = (als / 2.0).astype(np.float32)
        ahj = ah.reshape(64, 128).T.copy()                      # [128, 64]
        ahjo = ah[blk].reshape(8, 128).T.copy()                 # [128, 8]
        ahibc = np.broadcast_to(ah[blk], (128, NSH)).copy()
        cib = np.concatenate(
            [np.broadcast_to(cs[blk, d], (128, NSH)) for d in range(3)],
            axis=1).astype(np.float32).copy()                   # [128, 3072]
        ncjo = np.empty((128, 24), np.float32)
        for t in range(8):
            for d in range(3):
                ncjo[:, t * 3 + d] = -cs[core * NSH + t * 128:
                                         core * NSH + (t + 1) * 128, d]
        in_maps.append({
            "featj": featj, "feati": feati,
            "ahj": np.ascontiguousarray(ahj),
            "ahjo": np.ascontiguousarray(ahjo),
            "ahibc": ahibc, "cib": cib, "ncjo": ncjo,
            "latT": lats[blk].T.astype(np.float16).copy(),
            "w1t": W1.T.astype(np.float16).copy(),
            "w2t": W2.T.astype(np.float16).copy(),
            "b1c": b1.reshape(4, 128).T.astype(np.float32).copy(),
            "b2r": b2.reshape(1, DIM).astype(np.float16),
            "onescol": np.ones((1, 128), np.float16),
            "ones128": np.ones((128, 1), NP_BF16),
        })

    if "nc" not in _BUILT:
        _BUILT["nc"] = build_program()
    nc = _BUILT["nc"]
    res = run_bass_kernel_spmd(nc, in_maps, core_ids=list(range(NCORE)))

    num_all = np.zeros((N, DIM), np.float32)
    s_all = np.zeros(N, np.float32)
    for core in range(NCORE):
        blk = slice(core * NSH, (core + 1) * NSH)
        num_all[blk] = res.results[core]["num"]
        s_all[blk] = res.results[core]["srow"].T.reshape(-1)

    # host fix: add back exact K for suppressed straggler grid J* x I*
    need_rows = sorted(set().union(*Jstar)) if any(Jstar) else []
    if need_rows:
        lr = lats[need_rows]
        hh = lr @ W1.T + b1
        hh = (hh * 0.5 * (1.0 + _erf(hh / np.sqrt(2.0)))).astype(np.float32)
        msg_rows = (hh @ W2.T + b2).astype(np.float32)
        row_pos = {j: k for k, j in enumerate(need_rows)}
        for core in range(NCORE):
            J = sorted(Jstar[core])
            I = sorted(Istar[core])
            if not J or not I:
                continue
            Ig = np.array(I) + core * NSH
            d2c = ((c64[J][:, None, :] - c64[Ig][None, :, :]) ** 2).sum(-1)
            Dc = np.sqrt(d2c)
            aijc = (als[J].astype(np.float64)[:, None]
                    + als[Ig].astype(np.float64)[None, :]) * 0.5
            Kc = (Dc + EPS) ** (-aijc) * np.exp(-Dc / LAM)
            mrows = msg_rows[[row_pos[j] for j in J]]
            num_all[Ig] += (Kc.T @ mrows).astype(np.float32)
            s_all[Ig] += Kc.sum(0).astype(np.float32)

    out = lats + num_all / (s_all[:, None] + 1e-8)
    final = np.empty_like(out)
    final[perm] = out
    return final.astype(np.float32)



# revision 19
# speedup vs baseline: 1.0643x; 1.0643x over previous
"""Trainium2 Bass kernel for ContactDiffusion GNN message passing (v2).

out = latent + K_norm @ msg,  K = (D+eps)^(-alpha_ij) * exp(-D/12), row-normalized,
msg = MLP(latent).

Strategy (8 NeuronCores, SPMD single program, full inputs in / full output out):
 - Host: KD-sort points spatially; core c owns 1024 contiguous sorted rows.
 - Per core: K^T slab [8192 x 1024] via fp16-split Gram matmul (exact to
   ~1e-5 abs in d2); elementwise chain split across engines:
     ScalarE: Ln, Exp(d12), Exp(final)  (batched by activation-table set)
     VectorE: alpha add + multiply (fp16, 2x/4x DVE modes), per-tile column
              sums of K (row sums recovered on host via K symmetry)
     GpSimd:  tt = m + d12 add, diagonal zeroing (affine_select, own tiles)
 - All pairs with d2 < TSTRAG (symmetric global set V*) are suppressed on
   device via a rank-1 indicator feature and added back exactly on host.
 - Own-block (8 extra j-tiles per core) computed un-suppressed with the same
   Gram pass; diagonal zeroed by affine_select.
 - MLP sharded per core; msg exchanged via 4 chunked AllGathers overlapped
   with own-block compute.
"""

import math
import os
import sys
from contextlib import ExitStack

import numpy as np

sys.path.insert(0, "/opt/trn_rl_repo")

import ml_dtypes

import concourse.bass as bass
import concourse.tile as tile
from concourse import bacc, mybir
from concourse.bass_utils import run_bass_kernel_spmd

F32 = mybir.dt.float32
F16 = mybir.dt.float16
BF16 = mybir.dt.bfloat16
AF = mybir.ActivationFunctionType
ALU = mybir.AluOpType

NP_BF16 = ml_dtypes.bfloat16

N, DIM, NCORE = 8192, 512, 8
NSH = N // NCORE            # rows per core (1024)
EPS, LAM = 1e-4, 12.0
TSTRAG = 0.25               # d2 below this -> straggler (host-fixed exactly)
SUP = 1e3                   # suppressor feature magnitude (SUP^2 added to d2)
LN6 = math.log(6.0)
NT = 64                     # global j-tiles
NOWN = 8                    # own-block j-tiles
NIC = NSH // 128            # i-chunks (8)
NKD = DIM // 128            # MLP k-blocks (4)
NATLOG_EXP_SET = 6          # act_info.json index of natural_log_exp_and_others

_BUILT = {}


# ----------------------------------------------------------------------------
# device program (single SPMD program for all 8 cores)
# ----------------------------------------------------------------------------
def build_program(manual_tables=True):
    nc = bacc.Bacc("TRN2", target_bir_lowering=False, debug=False,
                   num_devices=NCORE)

    featj = nc.dram_tensor("featj", [18, N], F16, kind="ExternalInput").ap()
    feati = nc.dram_tensor("feati", [18, NSH], F16, kind="ExternalInput").ap()
    featjo = nc.dram_tensor("featjo", [18, NSH], F16, kind="ExternalInput").ap()
    ahj = nc.dram_tensor("ahj", [128, NT], F32, kind="ExternalInput").ap()
    ahjo = nc.dram_tensor("ahjo", [128, NOWN], F32, kind="ExternalInput").ap()
    ahibc = nc.dram_tensor("ahibc", [128, NSH], F16, kind="ExternalInput").ap()
    latT = nc.dram_tensor("latT", [DIM, NSH], F16, kind="ExternalInput").ap()
    w1t = nc.dram_tensor("w1t", [DIM, DIM], F16, kind="ExternalInput").ap()
    w2t = nc.dram_tensor("w2t", [DIM, DIM], F16, kind="ExternalInput").ap()
    b1c = nc.dram_tensor("b1c", [128, NKD], F32, kind="ExternalInput").ap()
    b2r = nc.dram_tensor("b2r", [1, DIM], F16, kind="ExternalInput").ap()
    onescol = nc.dram_tensor("onescol", [1, 128], F16, kind="ExternalInput").ap()

    num_out = nc.dram_tensor("num", [NSH, DIM], F32,
                             kind="ExternalOutput").ap()
    colsum_out = nc.dram_tensor("colsum", [128, NOWN + NT], F32,
                                kind="ExternalOutput").ap()

    with tile.TileContext(nc) as tc, ExitStack() as ctx:
        pers = ctx.enter_context(tc.tile_pool(name="pers", bufs=1))
        p_big = ctx.enter_context(tc.tile_pool(name="pbig", bufs=3, space="PSUM"))
        p_out = ctx.enter_context(tc.tile_pool(name="pout", bufs=2, space="PSUM"))
        fj_pool = ctx.enter_context(tc.tile_pool(name="fj", bufs=4))
        l_pool = ctx.enter_context(tc.tile_pool(name="lp", bufs=16))
        d12_pool = ctx.enter_context(tc.tile_pool(name="d12", bufs=9))
        al_pool = ctx.enter_context(tc.tile_pool(name="al", bufs=3))
        m_pool = ctx.enter_context(tc.tile_pool(name="mp", bufs=3))
        tt_pool = ctx.enter_context(tc.tile_pool(name="tt", bufs=7))
        kraw_pool = ctx.enter_context(tc.tile_pool(name="kraw", bufs=3))
        k_pool = ctx.enter_context(tc.tile_pool(name="kp", bufs=20))
        msg_pool = ctx.enter_context(tc.tile_pool(name="msgp", bufs=18))
        dram = ctx.enter_context(tc.tile_pool(name="dram", bufs=1, space="DRAM"))

        dma = nc.sync.dma_start

        def load_set(set_id):
            if not manual_tables:
                return
            ld = mybir.InstLoadActFuncSet(
                name=nc.get_next_instruction_name(), ins=[], outs=[],
                act_func_set_id=set_id)
            ld.engine = mybir.EngineType.Activation
            nc.scalar.add_instruction(ld)

        # ---- persistent SBUF loads (phase-A-critical first) ----
        featjo_sb = pers.tile([18, NSH], F16)
        dma(featjo_sb[:], featjo[:])
        feati_sb = pers.tile([18, NSH], F16)
        dma(feati_sb[:], feati[:])
        latT_sb = [pers.tile([128, NSH], F16, tag=f"latT{k}", name=f"latT{k}")
                   for k in range(NKD)]
        for k in range(NKD):
            dma(latT_sb[k][:], latT[k * 128:(k + 1) * 128, :])
        w1t_sb = [pers.tile([128, DIM], F16, tag=f"w1t{k}", name=f"w1t{k}")
                  for k in range(NKD)]
        for k in range(NKD):
            dma(w1t_sb[k][:], w1t[k * 128:(k + 1) * 128, :])
        b1c_sb = pers.tile([128, NKD], F32)
        dma(b1c_sb[:], b1c[:])
        ahibc_sb = pers.tile([128, NSH], F16)
        dma(ahibc_sb[:], ahibc[:])
        ahj_sb = pers.tile([128, NT], F32)
        dma(ahj_sb[:], ahj[:])
        ahjo_sb = pers.tile([128, NOWN], F32)
        dma(ahjo_sb[:], ahjo[:])
        w2t_sb = [pers.tile([128, DIM], F16, tag=f"w2t{k}", name=f"w2t{k}")
                  for k in range(NKD)]
        for k in range(NKD):
            dma(w2t_sb[k][:], w2t[k * 128:(k + 1) * 128, :])
        b2r_sb = pers.tile([1, DIM], F16)
        dma(b2r_sb[:], b2r[:])
        onescol_sb = pers.tile([1, 128], F16)
        dma(onescol_sb[:], onescol[:])

        acc = pers.tile([128, NIC * DIM], F32)
        nc.vector.memset(acc[:], 0.0)
        colsum_sb = pers.tile([128, NOWN + NT], F32)
        bias_ln6 = pers.tile([128, 1], F32)
        nc.gpsimd.memset(bias_ln6[:], -LN6)

        hT_sb = [pers.tile([128, NSH], F16, tag=f"hT{k}", name=f"hT{k}")
                 for k in range(NKD)]
        msgown_sb = [pers.tile([128, DIM], BF16, tag=f"mo{ic}", name=f"mo{ic}")
                     for ic in range(NIC)]
        msgown_ch = [dram.tile([2 * 128, DIM], BF16, tag=f"moch{c}",
                               name=f"moch{c}") for c in range(4)]
        msgall_ch = [dram.tile([16 * 128, DIM], BF16, tag=f"mach{c}",
                               name=f"mach{c}") for c in range(4)]

        load_set(NATLOG_EXP_SET)

        # ---- helpers ----
        def gram(lhs_sb, col0):
            """pairwise-d2 tile [128, NSH] in PSUM via feature matmul"""
            pd2 = p_big.tile([128, NSH], F32, tag="big", name="pd2")
            for half in range(2):
                hs = slice(half * 512, (half + 1) * 512)
                nc.tensor.matmul(pd2[:, hs],
                                 lhsT=lhs_sb[:, col0:col0 + 128],
                                 rhs=feati_sb[:, hs],
                                 start=True, stop=True)
            return pd2

        def gram_global(k):
            csrc, r = k % 8, k // 8
            gt = csrc * 8 + r
            fj = fj_pool.tile([18, 128], F16)
            dma(fj[:], featj[:, gt * 128:(gt + 1) * 128])
            return gram(fj, 0)

        def ln_pass(pd2):
            l = l_pool.tile([128, NSH], F16)
            nc.scalar.activation(l[:], pd2[:], AF.Ln)
            return l

        def exp_d12(l):
            d12 = d12_pool.tile([128, NSH], F16)
            nc.scalar.activation(d12[:], l[:], AF.Exp,
                                 bias=bias_ln6[:, 0:1], scale=0.5)
            return d12

        def vg_chain(l, d12, ah_col):
            """al = aij, m = aij*l, tt = m + d12 (V + G, fp16)"""
            al = al_pool.tile([128, NSH], F16)
            nc.vector.tensor_scalar_add(al[:], ahibc_sb[:], ah_col)
            m = m_pool.tile([128, NSH], F16)
            nc.vector.tensor_tensor(m[:], al[:], l[:], op=ALU.mult)
            t = tt_pool.tile([128, NSH], F16)
            nc.gpsimd.tensor_tensor(t[:], m[:], d12[:], op=ALU.add)
            return t

        def colsum(kt, col_idx):
            nc.vector.tensor_reduce(colsum_sb[:, col_idx:col_idx + 1], kt[:],
                                    axis=mybir.AxisListType.X, op=ALU.add)

        def contract_mm(group, gidx):
            """group: list of (ktile, msgtile); PE matmuls into 8 po tiles"""
            pos = []
            for ic in range(NIC):
                po = p_out.tile([128, DIM], F32, tag="out",
                                name=f"po{gidx}_{ic}")
                for i, (kt, mt) in enumerate(group):
                    nc.tensor.matmul(
                        po[:],
                        lhsT=kt[:, ic * 128:(ic + 1) * 128],
                        rhs=mt[:],
                        start=(i == 0), stop=(i == len(group) - 1))
                pos.append(po)
            return pos

        def acc_add(pos, ics):
            for ic in ics:
                asl = slice(ic * DIM, (ic + 1) * DIM)
                nc.vector.tensor_tensor(acc[:, asl], acc[:, asl],
                                        pos[ic][:], op=ALU.add)

        # ---- own tiles 0..1: gram + Ln (scalar warms up immediately) ----
        own_l = {}
        own_d12 = {}
        own_kt = {}
        for r in range(2):
            pd2 = gram(featjo_sb, r * 128)
            own_l[r] = ln_pass(pd2)

        # ---- phase A stage 1: hT = gelu(latent @ W1^T + b1) ----
        for mc in range(NKD):
            ph = p_big.tile([128, NSH], F32, tag="big", name="ph")
            for half in range(2):
                hs = slice(half * 512, (half + 1) * 512)
                for kb in range(NKD):
                    nc.tensor.matmul(
                        ph[:, hs],
                        lhsT=w1t_sb[kb][:, mc * 128:(mc + 1) * 128],
                        rhs=latT_sb[kb][:, hs],
                        start=(kb == 0), stop=(kb == NKD - 1))
            nc.scalar.activation(hT_sb[mc][:], ph[:], AF.Gelu,
                                 bias=b1c_sb[:, mc:mc + 1], scale=1.0)

        load_set(NATLOG_EXP_SET)

        # ---- own tiles 2..7 gram+Ln, then exps ----
        for r in range(2, NOWN):
            pd2 = gram(featjo_sb, r * 128)
            own_l[r] = ln_pass(pd2)
        for r in range(NOWN):
            own_d12[r] = exp_d12(own_l[r])
        own_tt = {r: vg_chain(own_l[r], own_d12[r], ahjo_sb[:, r:r + 1])
                  for r in range(NOWN)}
        own_kraw = {}
        for r in range(NOWN):
            own_kraw[r] = kraw_pool.tile([128, NSH], BF16, tag="kraw",
                                         name=f"kraw{r}")
            nc.scalar.activation(own_kraw[r][:], own_tt[r][:], AF.Exp,
                                 scale=-0.5)
        for r in range(NOWN):
            own_kt[r] = k_pool.tile([128, NSH], BF16, tag="kt",
                                    name=f"ktown{r}")
            nc.gpsimd.affine_select(
                own_kt[r][:], own_kraw[r][:], pattern=[[1, NSH]],
                compare_op=ALU.not_equal, fill=0.0,
                base=-(r * 128), channel_multiplier=-1)
        for r in range(NOWN):
            colsum(own_kt[r], r)

        # ---- phase A stage 2: msg = hT^T @ W2^T + b2 ; chunked AllGather ----
        for ic in range(NIC):
            pm = p_out.tile([128, DIM], F32, tag="out", name="pm")
            for kb in range(NKD):
                nc.tensor.matmul(
                    pm[:],
                    lhsT=hT_sb[kb][:, ic * 128:(ic + 1) * 128],
                    rhs=w2t_sb[kb][:],
                    start=(kb == 0), stop=False)
            nc.tensor.matmul(pm[:], lhsT=onescol_sb[:], rhs=b2r_sb[:],
                             start=False, stop=True)
            nc.vector.tensor_copy(msgown_sb[ic][:], pm[:])
            ch = ic // 2
            dma(msgown_ch[ch][(ic % 2) * 128:(ic % 2) * 128 + 128, :],
                msgown_sb[ic][:])
            if ic % 2 == 1:
                nc.gpsimd.collective_compute(
                    "AllGather", ALU.bypass,
                    ins=[msgown_ch[ch].opt()], outs=[msgall_ch[ch].opt()],
                    replica_groups=[list(range(NCORE))])

        # ---- slab pipeline: EW(g+1) grams/Ln  ->  C(g)  ->  EW(g+1) rest ----
        glob_kt = {}
        glob_mt = {}

        def ew_front(g):
            """grams + Ln for global group g (16 tiles), batched by 8"""
            ls = {}
            for k in range(g * 16, g * 16 + 16):
                pd2 = gram_global(k)
                ls[k] = ln_pass(pd2)
            return ls

        def ew_back_half(g, half, ls):
            """d12 + V/G chain + final exp + colsum + msg dma, 8 tiles"""
            ks = range(g * 16 + half * 8, g * 16 + half * 8 + 8)
            d12s = {k: exp_d12(ls[k]) for k in ks}
            tts = {k: vg_chain(ls[k], d12s[k],
                               ahj_sb[:, ((k % 8) * 8 + k // 8):
                                      ((k % 8) * 8 + k // 8) + 1])
                   for k in ks}
            for k in ks:
                glob_kt[k] = k_pool.tile([128, NSH], BF16, tag="kt",
                                         name=f"kt{k}")
                nc.scalar.activation(glob_kt[k][:], tts[k][:], AF.Exp,
                                     scale=-0.5)
            for k in ks:
                colsum(glob_kt[k], NOWN + k)
                csrc, r = k % 8, k // 8
                mt = msg_pool.tile([128, DIM], BF16)
                ch = r // 2
                row = csrc * 256 + (r % 2) * 128
                dma(mt[:], msgall_ch[ch][row:row + 128, :])
                glob_mt[k] = mt

        for g in range(4):
            ls = ew_front(g)
            if g == 0:
                prev = [(own_kt[r], msgown_sb[r]) for r in range(NOWN)]
            else:
                prev = [(glob_kt[k], glob_mt[k])
                        for k in range((g - 1) * 16, g * 16)]
            pos = contract_mm(prev, g)
            ew_back_half(g, 0, ls)
            acc_add(pos, range(0, 4))
            ew_back_half(g, 1, ls)
            acc_add(pos, range(4, 8))
        pos = contract_mm([(glob_kt[k], glob_mt[k])
                           for k in range(48, 64)], 4)
        acc_add(pos, range(0, 8))

        # ---- epilogue ----
        dma(colsum_out[:], colsum_sb[:])
        for ic in range(NIC):
            dma(num_out[ic * 128:(ic + 1) * 128, :],
                acc[:, ic * DIM:(ic + 1) * DIM])

    nc.compile()
    return nc


# ----------------------------------------------------------------------------
# host-side preprocessing
# ----------------------------------------------------------------------------
def _kdsort(coords, nblocks):
    def rec(idx, nb):
        if nb == 1:
            return [idx]
        pts = coords[idx]
        ax = int(np.argmax(pts.max(0) - pts.min(0)))
        order = np.argsort(pts[:, ax], kind="stable")
        half = len(idx) // 2
        return rec(idx[order[:half]], nb // 2) + rec(idx[order[half:]], nb // 2)

    return np.concatenate(rec(np.arange(coords.shape[0]), nblocks))


def _split16(x64):
    """fp16 hi/lo split of a float64 array (captures ~22 mantissa bits)"""
    hi = x64.astype(np.float16)
    lo = (x64 - hi.astype(np.float64)).astype(np.float16)
    return hi, lo


_erf = np.vectorize(math.erf)


def kernel(latent, coords, alpha, W1, b1, W2, b2):
    latent = np.asarray(latent, np.float32)
    coords = np.asarray(coords, np.float32)
    alpha = np.asarray(alpha, np.float32)
    W1 = np.asarray(W1, np.float32)
    b1 = np.asarray(b1, np.float32)
    W2 = np.asarray(W2, np.float32)
    b2 = np.asarray(b2, np.float32)

    perm = _kdsort(coords.astype(np.float64), 64)
    cs = coords[perm]
    als = alpha[perm]
    lats = latent[perm]
    c64 = cs.astype(np.float64)

    # ---- V*: symmetric global straggler set (all pairs d2 < TSTRAG) ----
    close = set()
    for i0 in range(0, N, 1024):
        blk = c64[i0:i0 + 1024]
        d2b = ((blk[:, None, :] - c64[None, :, :]) ** 2).sum(-1)
        d2b[np.arange(1024), np.arange(i0, i0 + 1024)] = np.inf
        ii, jj = np.nonzero(d2b < TSTRAG)
        close.update((i0 + ii).tolist())
        close.update(jj.tolist())
    VV = np.array(sorted(close), dtype=np.int64)
    g = np.zeros(N, np.float64)
    if len(VV):
        g[VV] = SUP

    # ---- fp16-split features ----
    r64 = (c64 ** 2).sum(-1)
    a64 = -2.0 * c64
    chj = [_split16(c64[:, d]) for d in range(3)]
    ahi = [_split16(a64[:, d]) for d in range(3)]
    rj = _split16(r64)
    ones_n = np.ones(N, np.float16)

    rows_j, rows_i_full = [], []
    for d in range(3):
        for (jp, ip) in [(chj[d][0], ahi[d][0]), (chj[d][0], ahi[d][1]),
                         (chj[d][1], ahi[d][0]), (chj[d][1], ahi[d][1])]:
            rows_j.append(jp)
            rows_i_full.append(ip)
    rows_j += [rj[0], rj[1], ones_n, ones_n, g.astype(np.float16)]
    rows_i_full += [ones_n, ones_n, rj[0], rj[1], g.astype(np.float16)]
    # row 17: mown (per-core) x SUP const
    featj_base = np.stack(rows_j).astype(np.float16)          # [17, N]
    feati_base = np.stack(rows_i_full).astype(np.float16)     # [17, N]

    ah = (als.astype(np.float64) / 2.0)
    ahj_all = ah.reshape(NT, 128).T.astype(np.float32).copy()  # [128, 64]

    in_maps = []
    for core in range(NCORE):
        blk = slice(core * NSH, (core + 1) * NSH)
        mown = np.zeros(N, np.float16)
        mown[blk] = SUP
        featj = np.concatenate([featj_base, mown[None, :]], axis=0)
        feati = np.concatenate(
            [feati_base[:, blk],
             np.full((1, NSH), SUP, np.float16)], axis=0)
        featjo = featj[:, blk].copy()
        featjo[17] = 0.0

        ahjo = ah[blk].reshape(NOWN, 128).T.astype(np.float32).copy()
        ahibc = np.broadcast_to(ah[blk].astype(np.float16),
                                (128, NSH)).copy()

        in_maps.append({
            "featj": np.ascontiguousarray(featj),
            "feati": np.ascontiguousarray(feati),
            "featjo": np.ascontiguousarray(featjo),
            "ahj": ahj_all,
            "ahjo": np.ascontiguousarray(ahjo),
            "ahibc": ahibc,
            "latT": lats[blk].T.astype(np.float16).copy(),
            "w1t": W1.T.astype(np.float16).copy(),
            "w2t": W2.T.astype(np.float16).copy(),
            "b1c": b1.reshape(NKD, 128).T.astype(np.float32).copy(),
            "b2r": b2.reshape(1, DIM).astype(np.float16),
            "onescol": np.ones((1, 128), np.float16),
        })

    if "nc" not in _BUILT:
        _BUILT["nc"] = build_program()
    nc = _BUILT["nc"]
    res = run_bass_kernel_spmd(nc, in_maps, core_ids=list(range(NCORE)))

    num_all = np.zeros((N, DIM), np.float32)
    s_all = np.zeros(N, np.float64)
    for core in range(NCORE):
        blk = slice(core * NSH, (core + 1) * NSH)
        num_all[blk] = res.results[core]["num"]
        colsum = res.results[core]["colsum"].astype(np.float64)  # [128, 72]
        # own tiles r=0..7: global j = core*NSH + r*128 + p
        for r in range(NOWN):
            s_all[core * NSH + r * 128: core * NSH + (r + 1) * 128] += \
                colsum[:, r]
        # global tiles k: gt = (k%8)*8 + k//8 ; j = gt*128 + p
        for k in range(NT):
            gt = (k % 8) * 8 + k // 8
            s_all[gt * 128:(gt + 1) * 128] += colsum[:, NOWN + k]

    # ---- host fix: exact K over the suppressed V* x V* grid ----
    if len(VV):
        lr = lats[VV].astype(np.float64)
        hh = lr @ W1.T.astype(np.float64) + b1.astype(np.float64)
        hh = hh * 0.5 * (1.0 + _erf(hh / np.sqrt(2.0)))
        msgV = hh @ W2.T.astype(np.float64) + b2.astype(np.float64)
        cV = c64[VV]
        d2V = ((cV[:, None, :] - cV[None, :, :]) ** 2).sum(-1)
        DV = np.sqrt(np.maximum(d2V, 0.0))
        aV = (als[VV].astype(np.float64)[:, None]
              + als[VV].astype(np.float64)[None, :]) * 0.5
        KV = (DV + EPS) ** (-aV) * np.exp(-DV / LAM)
        np.fill_diagonal(KV, 0.0)
        s_all[VV] += KV.sum(axis=1)
        num_all[VV] += (KV @ msgV).astype(np.float32)

    out = lats + num_all / (s_all[:, None].astype(np.float32) + 1e-8)
    final = np.empty_like(out)
    final[perm] = out
    return final.astype(np.float32)


# revision 23
# speedup vs baseline: 1.4103x; 1.3251x over previous
"""Trainium2 Bass kernel for ContactDiffusion GNN message passing (v2).

out = latent + K_norm @ msg,  K = (D+eps)^(-alpha_ij) * exp(-D/12), row-normalized,
msg = MLP(latent).

Strategy (8 NeuronCores, SPMD single program, full inputs in / full output out):
 - Host: KD-sort points spatially; core c owns 1024 contiguous sorted rows.
 - Per core: K^T slab [8192 x 1024] via fp16-split Gram matmul (exact to
   ~1e-5 abs in d2); elementwise chain split across engines:
     ScalarE: Ln, Exp(d12), Exp(final)  (batched by activation-table set)
     VectorE: alpha add + multiply (fp16, 2x/4x DVE modes), per-tile column
              sums of K (row sums recovered on host via K symmetry)
     GpSimd:  tt = m + d12 add, diagonal zeroing (affine_select, own tiles)
 - All pairs with d2 < TSTRAG (symmetric global set V*) are suppressed on
   device via a rank-1 indicator feature and added back exactly on host.
 - Own-block (8 extra j-tiles per core) computed un-suppressed with the same
   Gram pass; diagonal zeroed by affine_select.
 - MLP sharded per core; msg exchanged via 4 chunked AllGathers overlapped
   with own-block compute.
"""

import math
import os
import sys
from contextlib import ExitStack

import numpy as np

sys.path.insert(0, "/opt/trn_rl_repo")

import ml_dtypes

import concourse.bass as bass
import concourse.tile as tile
from concourse import bacc, mybir
from concourse.bass_utils import run_bass_kernel_spmd

F32 = mybir.dt.float32
F16 = mybir.dt.float16
BF16 = mybir.dt.bfloat16
AF = mybir.ActivationFunctionType
ALU = mybir.AluOpType

NP_BF16 = ml_dtypes.bfloat16

N, DIM, NCORE = 8192, 512, 8
NSH = N // NCORE            # rows per core (1024)
EPS, LAM = 1e-4, 12.0
TSTRAG = 0.25               # d2 below this -> straggler (host-fixed exactly)
SUP = 1e3                   # suppressor feature magnitude (SUP^2 added to d2)
LN6 = math.log(6.0)
NT = 64                     # global j-tiles
NOWN = 8                    # own-block j-tiles
NIC = NSH // 128            # i-chunks (8)
NKD = DIM // 128            # MLP k-blocks (4)
NATLOG_EXP_SET = 6          # act_info.json index of natural_log_exp_and_others

_BUILT = {}


# ----------------------------------------------------------------------------
# device program (single SPMD program for all 8 cores)
# ----------------------------------------------------------------------------
def build_program(manual_tables=True):
    nc = bacc.Bacc("TRN2", target_bir_lowering=False, debug=False,
                   num_devices=NCORE)

    featj = nc.dram_tensor("featj", [18, N], F16, kind="ExternalInput").ap()
    feati = nc.dram_tensor("feati", [18, NSH], F16, kind="ExternalInput").ap()
    featjo = nc.dram_tensor("featjo", [18, NSH], F16, kind="ExternalInput").ap()
    ahj = nc.dram_tensor("ahj", [128, NT], F32, kind="ExternalInput").ap()
    ahjo = nc.dram_tensor("ahjo", [128, NOWN], F32, kind="ExternalInput").ap()
    ahibc = nc.dram_tensor("ahibc", [128, NSH], F16, kind="ExternalInput").ap()
    latT = nc.dram_tensor("latT", [DIM, NSH], F16, kind="ExternalInput").ap()
    w1t = nc.dram_tensor("w1t", [DIM, DIM], F16, kind="ExternalInput").ap()
    w2t = nc.dram_tensor("w2t", [DIM, DIM], F16, kind="ExternalInput").ap()
    b1c = nc.dram_tensor("b1c", [128, NKD], F32, kind="ExternalInput").ap()
    b2r = nc.dram_tensor("b2r", [1, DIM], F16, kind="ExternalInput").ap()
    onescol = nc.dram_tensor("onescol", [1, 128], F16, kind="ExternalInput").ap()

    num_out = nc.dram_tensor("num", [NSH, DIM], F32,
                             kind="ExternalOutput").ap()
    colsum_out = nc.dram_tensor("colsum", [128, NOWN + NT], F32,
                                kind="ExternalOutput").ap()

    with tile.TileContext(nc) as tc, ExitStack() as ctx:
        pers = ctx.enter_context(tc.tile_pool(name="pers", bufs=1))
        p_big = ctx.enter_context(tc.tile_pool(name="pbig", bufs=2, space="PSUM"))
        p_out = ctx.enter_context(tc.tile_pool(name="pout", bufs=4, space="PSUM"))
        fj_pool = ctx.enter_context(tc.tile_pool(name="fj", bufs=4))
        lat_pool = ctx.enter_context(tc.tile_pool(name="latp", bufs=3))
        l_pool = ctx.enter_context(tc.tile_pool(name="lp", bufs=9))
        d12_pool = ctx.enter_context(tc.tile_pool(name="d12", bufs=4))
        al_pool = ctx.enter_context(tc.tile_pool(name="al", bufs=2))
        m_pool = ctx.enter_context(tc.tile_pool(name="mp", bufs=2))
        tt_pool = ctx.enter_context(tc.tile_pool(name="tt", bufs=4))
        kraw_pool = ctx.enter_context(tc.tile_pool(name="kraw", bufs=2))
        k_pool = ctx.enter_context(tc.tile_pool(name="kp", bufs=10))
        msg_pool = ctx.enter_context(tc.tile_pool(name="msgp", bufs=16))
        dram = ctx.enter_context(tc.tile_pool(name="dram", bufs=1, space="DRAM"))

        dma = nc.sync.dma_start

        def load_set(set_id):
            if not manual_tables:
                return
            ld = mybir.InstLoadActFuncSet(
                name=nc.get_next_instruction_name(), ins=[], outs=[],
                act_func_set_id=set_id)
            ld.engine = mybir.EngineType.Activation
            nc.scalar.add_instruction(ld)

        # ---- persistent SBUF loads (phase-A-critical first) ----
        featjo_sb = pers.tile([18, NSH], F16)
        dma(featjo_sb[:], featjo[:])
        feati_sb = pers.tile([18, NSH], F16)
        dma(feati_sb[:], feati[:])
        w1t_sb = [pers.tile([128, DIM], F16, tag=f"w1t{k}", name=f"w1t{k}")
                  for k in range(NKD)]
        for k in range(NKD):
            dma(w1t_sb[k][:], w1t[k * 128:(k + 1) * 128, :])
        b1c_sb = pers.tile([128, NKD], F32)
        dma(b1c_sb[:], b1c[:])
        ahibc_sb = pers.tile([128, NSH], F16)
        dma(ahibc_sb[:], ahibc[:])
        ahj_sb = pers.tile([128, NT], F32)
        dma(ahj_sb[:], ahj[:])
        ahjo_sb = pers.tile([128, NOWN], F32)
        dma(ahjo_sb[:], ahjo[:])
        w2t_sb = [pers.tile([128, DIM], F16, tag=f"w2t{k}", name=f"w2t{k}")
                  for k in range(NKD)]
        for k in range(NKD):
            dma(w2t_sb[k][:], w2t[k * 128:(k + 1) * 128, :])
        b2r_sb = pers.tile([1, DIM], F16)
        dma(b2r_sb[:], b2r[:])
        onescol_sb = pers.tile([1, 128], F16)
        dma(onescol_sb[:], onescol[:])

        acc = pers.tile([128, NIC * DIM], F32)
        nc.vector.memset(acc[:], 0.0)
        colsum_sb = pers.tile([128, NOWN + NT], F32)
        bias_ln6 = pers.tile([128, 1], F32)
        nc.gpsimd.memset(bias_ln6[:], -LN6)

        hT_sb = [pers.tile([128, NSH], F16, tag=f"hT{k}", name=f"hT{k}")
                 for k in range(NKD)]
        msgown_sb = [pers.tile([128, DIM], BF16, tag=f"mo{ic}", name=f"mo{ic}")
                     for ic in range(NIC)]
        msgown_ch = [dram.tile([2 * 128, DIM], BF16, tag=f"moch{c}",
                               name=f"moch{c}") for c in range(4)]
        msgall_ch = [dram.tile([16 * 128, DIM], BF16, tag=f"mach{c}",
                               name=f"mach{c}") for c in range(4)]

        load_set(NATLOG_EXP_SET)

        # ---- helpers (pair-merged elementwise at FD=2048) ----
        def gram(lhs_sb, col0):
            pd2 = p_big.tile([128, NSH], F32, tag="big", name="pd2")
            for half in range(2):
                hs = slice(half * 512, (half + 1) * 512)
                nc.tensor.matmul(pd2[:, hs],
                                 lhsT=lhs_sb[:, col0:col0 + 128],
                                 rhs=feati_sb[:, hs],
                                 start=True, stop=True)
            return pd2

        def gt_of(k):
            return (k % 8) * 8 + k // 8

        def gram_global(k):
            gt = gt_of(k)
            fj = fj_pool.tile([18, 128], F16)
            dma(fj[:], featj[:, gt * 128:(gt + 1) * 128])
            return gram(fj, 0)

        def ln_to(l2, sub, pd2):
            nc.scalar.activation(l2[:, sub * NSH:(sub + 1) * NSH], pd2[:],
                                 AF.Ln)

        def pair_chain(l2, ah_cols, name, pool=None):
            """d12/al/m/tt/final-exp on a [128, 2*NSH] pair; returns kt2"""
            pool = pool or k_pool
            d12 = d12_pool.tile([128, 2 * NSH], F16, tag="d12",
                                name=f"d12{name}")
            nc.scalar.activation(d12[:], l2[:], AF.Exp,
                                 bias=bias_ln6[:, 0:1], scale=0.5)
            al = al_pool.tile([128, 2 * NSH], F16, tag="al", name=f"al{name}")
            for sub in range(2):
                nc.vector.tensor_scalar_add(
                    al[:, sub * NSH:(sub + 1) * NSH], ahibc_sb[:],
                    ah_cols[sub])
            m = m_pool.tile([128, 2 * NSH], F16, tag="m", name=f"m{name}")
            nc.vector.tensor_tensor(m[:], al[:], l2[:], op=ALU.mult)
            t = tt_pool.tile([128, 2 * NSH], F16, tag="tt", name=f"tt{name}")
            nc.vector.tensor_tensor(t[:], m[:], d12[:], op=ALU.add)
            kt2 = pool.tile([128, 2 * NSH], BF16, tag="kt", name=f"kt{name}")
            nc.scalar.activation(kt2[:], t[:], AF.Exp, scale=-0.5)
            return kt2

        def colsum(kt2, sub, col_idx):
            nc.vector.tensor_reduce(
                colsum_sb[:, col_idx:col_idx + 1],
                kt2[:, sub * NSH:(sub + 1) * NSH],
                axis=mybir.AxisListType.X, op=ALU.add)

        def contract_mm(group, gidx):
            """group: list of (kt2, sub, msgtile); matmuls into 8 po tiles"""
            pos = []
            for ic in range(NIC):
                po = p_out.tile([128, DIM], F32, tag="out",
                                name=f"po{gidx}_{ic}")
                for i, (kt2, sub, mt) in enumerate(group):
                    c0 = sub * NSH + ic * 128
                    nc.tensor.matmul(
                        po[:], lhsT=kt2[:, c0:c0 + 128], rhs=mt[:],
                        start=(i == 0), stop=(i == len(group) - 1))
                pos.append(po)
            return pos

        def acc_add(pos, ics):
            for ic in ics:
                asl = slice(ic * DIM, (ic + 1) * DIM)
                nc.vector.tensor_tensor(acc[:, asl], acc[:, asl],
                                        pos[ic][:], op=ALU.add)

        # ---- own pair 0 grams + Ln (scalar warms up immediately) ----
        own_l2 = {p: l_pool.tile([128, 2 * NSH], F16, tag="l2",
                                 name=f"l2own{p}") for p in range(4)}
        own_kt2 = {}
        for r in range(2):
            ln_to(own_l2[0], r % 2, gram(featjo_sb, r * 128))

        # ---- phase A stage 1: hT = gelu(latent @ W1^T + b1) ----
        for mc in range(NKD):
            ph = p_big.tile([128, NSH], F32, tag="big", name="ph")
            for half in range(2):
                hs = slice(half * 512, (half + 1) * 512)
                for kb in range(NKD):
                    lt = lat_pool.tile([128, 512], F16)
                    dma(lt[:], latT[kb * 128:(kb + 1) * 128,
                                    half * 512:(half + 1) * 512])
                    nc.tensor.matmul(
                        ph[:, hs],
                        lhsT=w1t_sb[kb][:, mc * 128:(mc + 1) * 128],
                        rhs=lt[:],
                        start=(kb == 0), stop=(kb == NKD - 1))
            nc.scalar.activation(hT_sb[mc][:], ph[:], AF.Gelu,
                                 bias=b1c_sb[:, mc:mc + 1], scale=1.0)

        load_set(NATLOG_EXP_SET)

        # ---- own pairs 1..3 grams + Ln, then pair chains ----
        for r in range(2, NOWN):
            ln_to(own_l2[r // 2], r % 2, gram(featjo_sb, r * 128))
        for p in range(4):
            kraw2 = pair_chain(own_l2[p],
                               [ahjo_sb[:, 2 * p:2 * p + 1],
                                ahjo_sb[:, 2 * p + 1:2 * p + 2]], f"ow{p}",
                               pool=kraw_pool)
            kt2 = k_pool.tile([128, 2 * NSH], BF16, tag="kt",
                              name=f"ktow{p}")
            own_kt2[p] = kt2
            for sub in range(2):
                r = 2 * p + sub
                nc.gpsimd.affine_select(
                    kt2[:, sub * NSH:(sub + 1) * NSH],
                    kraw2[:, sub * NSH:(sub + 1) * NSH],
                    pattern=[[1, NSH]],
                    compare_op=ALU.not_equal, fill=0.0,
                    base=-(r * 128), channel_multiplier=-1)
            for sub in range(2):
                colsum(own_kt2[p], sub, 2 * p + sub)

        # ---- phase A stage 2: msg = hT^T @ W2^T + b2 ; chunked AllGather ----
        for ic in range(NIC):
            pm = p_out.tile([128, DIM], F32, tag="out", name="pm")
            for kb in range(NKD):
                nc.tensor.matmul(
                    pm[:],
                    lhsT=hT_sb[kb][:, ic * 128:(ic + 1) * 128],
                    rhs=w2t_sb[kb][:],
                    start=(kb == 0), stop=False)
            nc.tensor.matmul(pm[:], lhsT=onescol_sb[:], rhs=b2r_sb[:],
                             start=False, stop=True)
            nc.vector.tensor_copy(msgown_sb[ic][:], pm[:])
            ch = ic // 2
            dma(msgown_ch[ch][(ic % 2) * 128:(ic % 2) * 128 + 128, :],
                msgown_sb[ic][:])
            if ic % 2 == 1:
                nc.gpsimd.collective_compute(
                    "AllGather", ALU.bypass,
                    ins=[msgown_ch[ch].opt()], outs=[msgall_ch[ch].opt()],
                    replica_groups=[list(range(NCORE))])

        # ---- slab pipeline ----
        glob_kt2 = {}
        glob_mt = {}

        def ew_front(g):
            l2s = {}
            for p in range(8):
                l2s[p] = l_pool.tile([128, 2 * NSH], F16, tag="l2",
                                     name=f"l2g{g}p{p}")
                for sub in range(2):
                    k = g * 16 + 2 * p + sub
                    ln_to(l2s[p], sub, gram_global(k))
            return l2s

        def ew_back_half(g, half, l2s):
            for p in range(half * 4, half * 4 + 4):
                k0 = g * 16 + 2 * p
                kt2 = pair_chain(
                    l2s[p],
                    [ahj_sb[:, gt_of(k0):gt_of(k0) + 1],
                     ahj_sb[:, gt_of(k0 + 1):gt_of(k0 + 1) + 1]],
                    f"g{g}p{p}")
                glob_kt2[k0 // 2] = kt2
                for sub in range(2):
                    k = k0 + sub
                    colsum(kt2, sub, NOWN + k)
                    csrc, r = k % 8, k // 8
                    mt = msg_pool.tile([128, DIM], BF16)
                    dma(mt[:], msgall_ch[r // 2][csrc * 256 + (r % 2) * 128:
                                                 csrc * 256 + (r % 2) * 128
                                                 + 128, :])
                    glob_mt[k] = mt

        def group_pairs(g):
            return [(glob_kt2[(g * 16 + 2 * p) // 2], sub,
                     glob_mt[g * 16 + 2 * p + sub])
                    for p in range(8) for sub in range(2)]

        for g in range(4):
            l2s = ew_front(g)
            if g == 0:
                prev = [(own_kt2[r // 2], r % 2, msgown_sb[r])
                        for r in range(NOWN)]
            else:
                prev = group_pairs(g - 1)
            pos = contract_mm(prev, g)
            ew_back_half(g, 0, l2s)
            acc_add(pos, range(0, 4))
            ew_back_half(g, 1, l2s)
            acc_add(pos, range(4, 8))
        pos = contract_mm(group_pairs(3), 4)
        acc_add(pos, range(0, 8))

        # ---- epilogue ----
        dma(colsum_out[:], colsum_sb[:])
        for ic in range(NIC):
            dma(num_out[ic * 128:(ic + 1) * 128, :],
                acc[:, ic * DIM:(ic + 1) * DIM])

    nc.compile()
    return nc


# ----------------------------------------------------------------------------
# host-side preprocessing
# ----------------------------------------------------------------------------
def _kdsort(coords, nblocks):
    def rec(idx, nb):
        if nb == 1:
            return [idx]
        pts = coords[idx]
        ax = int(np.argmax(pts.max(0) - pts.min(0)))
        order = np.argsort(pts[:, ax], kind="stable")
        half = len(idx) // 2
        return rec(idx[order[:half]], nb // 2) + rec(idx[order[half:]], nb // 2)

    return np.concatenate(rec(np.arange(coords.shape[0]), nblocks))


def _split16(x64):
    """fp16 hi/lo split of a float64 array (captures ~22 mantissa bits)"""
    hi = x64.astype(np.float16)
    lo = (x64 - hi.astype(np.float64)).astype(np.float16)
    return hi, lo


_erf = np.vectorize(math.erf)


def kernel(latent, coords, alpha, W1, b1, W2, b2):
    latent = np.asarray(latent, np.float32)
    coords = np.asarray(coords, np.float32)
    alpha = np.asarray(alpha, np.float32)
    W1 = np.asarray(W1, np.float32)
    b1 = np.asarray(b1, np.float32)
    W2 = np.asarray(W2, np.float32)
    b2 = np.asarray(b2, np.float32)

    perm = _kdsort(coords.astype(np.float64), 64)
    cs = coords[perm]
    als = alpha[perm]
    lats = latent[perm]
    c64 = cs.astype(np.float64)

    # ---- V*: symmetric global straggler set (all pairs d2 < TSTRAG) ----
    close = set()
    for i0 in range(0, N, 1024):
        blk = c64[i0:i0 + 1024]
        d2b = ((blk[:, None, :] - c64[None, :, :]) ** 2).sum(-1)
        d2b[np.arange(1024), np.arange(i0, i0 + 1024)] = np.inf
        ii, jj = np.nonzero(d2b < TSTRAG)
        close.update((i0 + ii).tolist())
        close.update(jj.tolist())
    VV = np.array(sorted(close), dtype=np.int64)
    g = np.zeros(N, np.float64)
    if len(VV):
        g[VV] = SUP

    # ---- fp16-split features ----
    r64 = (c64 ** 2).sum(-1)
    a64 = -2.0 * c64
    chj = [_split16(c64[:, d]) for d in range(3)]
    ahi = [_split16(a64[:, d]) for d in range(3)]
    rj = _split16(r64)
    ones_n = np.ones(N, np.float16)

    rows_j, rows_i_full = [], []
    for d in range(3):
        for (jp, ip) in [(chj[d][0], ahi[d][0]), (chj[d][0], ahi[d][1]),
                         (chj[d][1], ahi[d][0]), (chj[d][1], ahi[d][1])]:
            rows_j.append(jp)
            rows_i_full.append(ip)
    rows_j += [rj[0], rj[1], ones_n, ones_n, g.astype(np.float16)]
    rows_i_full += [ones_n, ones_n, rj[0], rj[1], g.astype(np.float16)]
    # row 17: mown (per-core) x SUP const
    featj_base = np.stack(rows_j).astype(np.float16)          # [17, N]
    feati_base = np.stack(rows_i_full).astype(np.float16)     # [17, N]

    ah = (als.astype(np.float64) / 2.0)
    ahj_all = ah.reshape(NT, 128).T.astype(np.float32).copy()  # [128, 64]

    in_maps = []
    for core in range(NCORE):
        blk = slice(core * NSH, (core + 1) * NSH)
        mown = np.zeros(N, np.float16)
        mown[blk] = SUP
        featj = np.concatenate([featj_base, mown[None, :]], axis=0)
        feati = np.concatenate(
            [feati_base[:, blk],
             np.full((1, NSH), SUP, np.float16)], axis=0)
        featjo = featj[:, blk].copy()
        featjo[17] = 0.0

        ahjo = ah[blk].reshape(NOWN, 128).T.astype(np.float32).copy()
        ahibc = np.broadcast_to(ah[blk].astype(np.float16),
                                (128, NSH)).copy()

        in_maps.append({
            "featj": np.ascontiguousarray(featj),
            "feati": np.ascontiguousarray(feati),
            "featjo": np.ascontiguousarray(featjo),
            "ahj": ahj_all,
            "ahjo": np.ascontiguousarray(ahjo),
            "ahibc": ahibc,
            "latT": lats[blk].T.astype(np.float16).copy(),
            "w1t": W1.T.astype(np.float16).copy(),
            "w2t": W2.T.astype(np.float16).copy(),
            "b1c": b1.reshape(NKD, 128).T.astype(np.float32).copy(),
            "b2r": b2.reshape(1, DIM).astype(np.float16),
            "onescol": np.ones((1, 128), np.float16),
        })

    if "nc" not in _BUILT:
        _BUILT["nc"] = build_program()
    nc = _BUILT["nc"]
    res = run_bass_kernel_spmd(nc, in_maps, core_ids=list(range(NCORE)))

    num_all = np.zeros((N, DIM), np.float32)
    s_all = np.zeros(N, np.float64)
    for core in range(NCORE):
        blk = slice(core * NSH, (core + 1) * NSH)
        num_all[blk] = res.results[core]["num"]
        colsum = res.results[core]["colsum"].astype(np.float64)  # [128, 72]
        # own tiles r=0..7: global j = core*NSH + r*128 + p
        for r in range(NOWN):
            s_all[core * NSH + r * 128: core * NSH + (r + 1) * 128] += \
                colsum[:, r]
        # global tiles k: gt = (k%8)*8 + k//8 ; j = gt*128 + p
        for k in range(NT):
            gt = (k % 8) * 8 + k // 8
            s_all[gt * 128:(gt + 1) * 128] += colsum[:, NOWN + k]

    # ---- host fix: exact K over the suppressed V* x V* grid ----
    if len(VV):
        lr = lats[VV].astype(np.float64)
        hh = lr @ W1.T.astype(np.float64) + b1.astype(np.float64)
        hh = hh * 0.5 * (1.0 + _erf(hh / np.sqrt(2.0)))
        msgV = hh @ W2.T.astype(np.float64) + b2.astype(np.float64)
        cV = c64[VV]
        d2V = ((cV[:, None, :] - cV[None, :, :]) ** 2).sum(-1)
        DV = np.sqrt(np.maximum(d2V, 0.0))
        aV = (als[VV].astype(np.float64)[:, None]
              + als[VV].astype(np.float64)[None, :]) * 0.5
        KV = (DV + EPS) ** (-aV) * np.exp(-DV / LAM)
        np.fill_diagonal(KV, 0.0)
        s_all[VV] += KV.sum(axis=1)
        num_all[VV] += (KV @ msgV).astype(np.float32)

    out = lats + num_all / (s_all[:, None].astype(np.float32) + 1e-8)
    final = np.empty_like(out)
    final[perm] = out
    return final.astype(np.float32)
